# revision 4
# baseline (speedup 1.0000x reference)
"""Contrastive loss kernel for Trainium2 (8 NeuronCores, SPMD via bass).

Strategy (v2 — fp8 DoubleRow + first-order log expansion):
  * Host sorts the batch by label (loss is invariant under a joint row/col
    permutation); same-label columns become one contiguous range per label.
  * Launch A (data-parallel over rows): host supplies embT (k-tile-major
    f32, cast to fp8 in the DMA) and fp8 weights (x64). fp8 DoubleRow
    matmuls (K=256 per instruction at 2 cols/cycle) compute
    psE = 64*(emb @ W.T) + 64*b (bias via a k=1 fp8 matmul), norms via a
    ones-matmul over bf16 squares, and emit ent = 16*normalize(e).T as fp8
    plus S = en @ lnT (fp8 DoubleRow, x256 scale).
  * Launch B: all cores load the assembled full ent [256, 8192] (fp8,
    chunk-grouped [g][m][2048]). Each core owns up to B label-pure row
    blocks (<=128 rows). Per block: 16 fp8 DoubleRow matmuls produce the
    [128, 8192] cosine row in four 2048-col PSUM chunks; one ACT Exp per
    chunk (accum_out -> row sums). The same-label range is a zero-padded
    fp8 copy (width W_s): 2 DoubleRow matmuls + one ACT Exp accum give ss;
    a DVE reduce over its PSUM gives csr = sum(C).
  * exp(C)/negsum ~ 1e-4, so ln(negsum + y) = ln(negsum) + y/negsum to
    first order (error ~1e-9 on the final loss). The inter-sample term
    collapses to
      term = ((BS-1)*ln(negsum) + (ss + BS - W_s - e)/negsum + 1 - csr)*mask
    (zero-pad corrections fold into the global constant BS - W_s - e).
    Everything except the (BS-1)*ln(negsum) part is assembled per block on
    DVE behind the Exp stream; one batched ACT Ln on [128, B] per core plus
    one fused DVE op per block finishes the terms (a single Exp->Ln
    activation-table switch per core).
  * Host: inter = sum(terms)/bs^2; l1/l2 finalized from S in float64.
"""

import math
import os

import ml_dtypes
import numpy as np

os.environ.setdefault("NEURON_RT_VIRTUAL_CORE_SIZE", "1")

import concourse.bass as bass
import concourse.mybir as mybir
from concourse import bacc
import concourse.tile as tile
from concourse.bass_utils import run_bass_kernel_spmd

BS = 8192
D_IN = 1024
D_EMB = 256
L = 10
NC = 8
P = 128
RPC = BS // NC          # rows per core in launch A (1024)
RT = RPC // P           # 128-row tiles per core (8)
KT = D_IN // P          # k tiles (8)
KM = D_EMB // P         # emb-dim partition chunks (2)
CW = 2048               # psum chunk width (4 banks)
NG = BS // CW           # chunks per cosine row (4)

F32 = mybir.dt.float32
BF16 = mybir.dt.bfloat16
F8 = mybir.dt.float8e4
BF16_NP = ml_dtypes.bfloat16
F8_NP = ml_dtypes.float8_e4m3
W_SCALE = 64.0          # fp8 weight scale in launch A
EN_SCALE = 16.0         # ent = EN_SCALE * normalize(e).T
CSC = EN_SCALE * EN_SCALE   # cosine-psum scale (256)
AX = mybir.AxisListType.X
AF = mybir.ActivationFunctionType
DR = mybir.MatmulPerfMode.DoubleRow
MUL = mybir.AluOpType.mult
ADD = mybir.AluOpType.add

# Results of the last kernel() call (for test.py introspection/timing).
LAST = {}


# --------------------------------------------------------------------------
# Launch A: per-core transform  -> ent_out[128, KM*RPC] f8 (16*en.T), s_out
# --------------------------------------------------------------------------
def build_launch_a():
    nc = bacc.Bacc("TRN2", target_bir_lowering=False, debug=False, num_devices=NC)
    embt_d = nc.dram_tensor("embt", [P, KT * RPC], F32, kind="ExternalInput")
    wt_d = nc.dram_tensor("wt", [P, KT * D_EMB], F8, kind="ExternalInput")
    brow_d = nc.dram_tensor("brow", [1, KM * P], F8, kind="ExternalInput")
    lnt_d = nc.dram_tensor("lnt", [P, KM * L], F8, kind="ExternalInput")
    ent_d = nc.dram_tensor("ent_out", [P, KM * RPC], F8, kind="ExternalOutput")
    s_d = nc.dram_tensor("s_out", [P, RT * L], F32, kind="ExternalOutput")

    with tile.TileContext(nc) as tc:
        with (
            tc.tile_pool(name="const", bufs=1) as cpool,
            tc.tile_pool(name="big", bufs=1) as big_pool,
            tc.tile_pool(name="ps", bufs=1, space="PSUM") as ps_pool,
        ):
            embt_sb = big_pool.tile([P, KT, RPC], F8)
            # first embt chunk on the SWDGE queue before anything else; the
            # small fp8 constants ride the SP HWDGE queue in parallel
            nc.gpsimd.dma_start(
                embt_sb[:, 0:2, :], embt_d.ap()[:, 0:2 * RPC])
            wt_sb = cpool.tile([P, KT, D_EMB], F8)
            nc.sync.dma_start(wt_sb[:, :, :], wt_d.ap())
            brow_sb = cpool.tile([1, KM, P], F8)
            nc.sync.dma_start(brow_sb[:, :, :], brow_d.ap())
            lnt_sb = cpool.tile([P, KM, L], F8)
            nc.sync.dma_start(lnt_sb[:, :, :], lnt_d.ap())
            ones_row = cpool.tile([1, 512], F8)
            nc.vector.memset(ones_row[:], 1.0)
            ones_col = cpool.tile([P, 1], BF16)
            nc.vector.memset(ones_col[:], 1.0)
            # psb = (EN_SCALE / W_SCALE) / norm, constant folded into the
            # partition-broadcast matmul below
            onesq = cpool.tile([1, P], BF16)
            nc.vector.memset(onesq[:], EN_SCALE / W_SCALE)
            # dummy sqrt up front pins the sqrt_and_others act table (which
            # also serves Square and Copy) so no reload lands mid-chain
            dumm = cpool.tile([1, 1], F32)
            nc.vector.memset(dumm[:], 1.0)
            nc.scalar.sqrt(dumm[:], dumm[:])

            psE = [ps_pool.tile([P, RPC], F32, name=f"psE{m}") for m in range(KM)]

            # stream k-pairs: DMA chunk kk (f32->f8 cast in flight), then
            # accumulate fp8 DoubleRow matmuls (K=256 per instruction)
            for kk in range(KT // 2):
                if kk > 0:
                    nc.gpsimd.dma_start(
                        embt_sb[:, 2 * kk:2 * kk + 2, :],
                        embt_d.ap()[:, 2 * kk * RPC:(2 * kk + 2) * RPC],
                    )
                for m in range(KM):
                    for n in range(RPC // 512):
                        nc.tensor.matmul(
                            psE[m][:, n * 512:(n + 1) * 512],
                            wt_sb[:, 2 * kk:2 * kk + 2, m * P:(m + 1) * P],
                            embt_sb[:, 2 * kk:2 * kk + 2, n * 512:(n + 1) * 512],
                            start=(kk == 0),
                            stop=False,
                            perf_mode=DR,
                        )
            # bias rows (k=1 fp8): psE = W_SCALE * (emb @ W.T + b)
            for m in range(KM):
                for n in range(RPC // 512):
                    nc.tensor.matmul(
                        psE[m][:, n * 512:(n + 1) * 512],
                        brow_sb[:, m, :],
                        ones_row[:],
                        start=False,
                        stop=True,
                    )

            # norm chain, split into 512-col halves so ACT/PE/DVE pipeline:
            # esq = (psE/W_SCALE)^2 (bf16) -> psN = colsum -> sqrt -> 1/x ->
            # psB = (EN_SCALE/W_SCALE)/norm broadcast -> ent = psE * psB (f8)
            NH = RPC // 512
            esq = big_pool.tile([P, KM * RPC], BF16)
            rn = big_pool.tile([1, RPC], F32)
            rni = big_pool.tile([1, RPC], BF16)
            ent_sb = big_pool.tile([P, KM, RPC], F8)
            psB = [ps_pool.tile([P, 512], F32, name=f"psB{n}") for n in range(NH)]
            psN = [ps_pool.tile([1, 512], F32, tag="aux", bufs=2, name=f"psN{n}")
                   for n in range(NH)]
            sbB = [big_pool.tile([P, 512], BF16, name=f"sbB{n}") for n in range(NH)]
            # phase-ordered so no engine stalls behind a later dependency:
            # ACT: squares -> sqrts -> psB copies; PE: psN -> psB; DVE:
            # recips -> ent muls
            for n in range(NH):
                for m in range(KM):
                    nc.scalar.activation(
                        esq[:, m * RPC + n * 512: m * RPC + (n + 1) * 512],
                        psE[m][:, n * 512:(n + 1) * 512],
                        AF.Square, scale=1.0 / W_SCALE,
                    )
            for n in range(NH):
                for m in range(KM):
                    nc.tensor.matmul(
                        psN[n][:],
                        ones_col[:],
                        esq[:, m * RPC + n * 512: m * RPC + (n + 1) * 512],
                        start=(m == 0),
                        stop=(m == KM - 1),
                    )
            for n in range(NH):
                nc.scalar.sqrt(rn[:, n * 512:(n + 1) * 512], psN[n][:])
            with nc.allow_low_precision(reason="1/norm feeds fp8 output"):
                for n in range(NH):
                    nc.vector.reciprocal(
                        rni[:, n * 512:(n + 1) * 512],
                        rn[:, n * 512:(n + 1) * 512])
            for n in range(NH):
                nc.tensor.matmul(
                    psB[n][:], onesq[:], rni[:, n * 512:(n + 1) * 512],
                    start=True, stop=True)
            for n in range(NH):
                # DVE can read only one PSUM operand: stage psB in SBUF
                nc.scalar.activation(sbB[n][:], psB[n][:], AF.Copy)
            for n in range(NH):
                for m in range(KM):
                    nc.vector.tensor_mul(
                        ent_sb[:, m, n * 512:(n + 1) * 512],
                        psE[m][:, n * 512:(n + 1) * 512], sbB[n][:])
            nc.sync.dma_start(ent_d.ap(), ent_sb[:, :, :])

            # S = en @ lnT (fp8 DoubleRow; psS = CSC * S)
            psS = ps_pool.tile([P, RT * L], F32, tag="aux", bufs=2)
            for r in range(RT):
                nc.tensor.matmul(
                    psS[:, r * L:(r + 1) * L],
                    ent_sb[:, 0:KM, r * P:(r + 1) * P],
                    lnt_sb[:, 0:KM, :],
                    start=True,
                    stop=True,
                    perf_mode=DR,
                )
            s_sb = big_pool.tile([P, RT * L], F32)
            nc.vector.tensor_copy(s_sb[:], psS[:])
            nc.gpsimd.dma_start(s_d.ap(), s_sb[:])

    nc.compile()
    return nc


# --------------------------------------------------------------------------
# Launch B: B label-pure block slots of the inter-sample loss per core
# --------------------------------------------------------------------------
def build_launch_b(B, W_s):
    WH = W_s // 512
    C0 = float(BS - W_s - math.e)
    nc = bacc.Bacc("TRN2", target_bir_lowering=False, debug=False, num_devices=NC)
    ent_d = nc.dram_tensor("ent", [P, NG * KM * CW], F8, kind="ExternalInput")
    lhst_d = nc.dram_tensor("lhst", [P, B * KM * P], F8, kind="ExternalInput")
    rs_d = nc.dram_tensor("rsame", [P, B * KM * W_s], F8, kind="ExternalInput")
    meta_d = nc.dram_tensor("meta", [P, 3 * B], F32, kind="ExternalInput")
    terms_d = nc.dram_tensor("terms", [P, B], F32, kind="ExternalOutput")

    with tile.TileContext(nc) as tc:
        with (
            tc.tile_pool(name="inp", bufs=1) as inp_pool,
            tc.tile_pool(name="scr", bufs=2) as scr_pool,
            tc.tile_pool(name="sml", bufs=2) as sml_pool,
            tc.tile_pool(name="fin", bufs=1) as fin_pool,
            tc.tile_pool(name="psm", bufs=2, space="PSUM") as psm_pool,
        ):
            ent_sb = inp_pool.tile([P, NG * KM, CW], F8)
            lhst_sb = inp_pool.tile([P, B * KM, P], F8)
            rs_sb = inp_pool.tile([P, B * KM, W_s], F8)
            meta_sb = inp_pool.tile([P, 3 * B], F32)
            # SP HWDGE queue in dependency order: block-0 lhs and the ent
            # chunks first; rsame streams behind in per-2-block pieces on
            # the SWDGE queue so it never overtakes the ent chunks; meta is
            # only needed at block-0 wrap-up
            nc.sync.dma_start(lhst_sb[:, 0:KM, :], lhst_d.ap()[:, 0:KM * P])
            for g in range(NG):
                nc.sync.dma_start(
                    ent_sb[:, g * KM:(g + 1) * KM, :],
                    ent_d.ap()[:, g * KM * CW:(g + 1) * KM * CW],
                )
            if B > 1:
                nc.sync.dma_start(
                    lhst_sb[:, KM:B * KM, :], lhst_d.ap()[:, KM * P:])
            nc.sync.dma_start(meta_sb[:], meta_d.ap())
            for b0 in range(0, B, 2):
                b1 = min(b0 + 2, B)
                nc.gpsimd.dma_start(
                    rs_sb[:, b0 * KM:b1 * KM, :],
                    rs_d.ap()[:, b0 * KM * W_s:b1 * KM * W_s])
            pad_sb = meta_sb[:, 0:B]
            mask_sb = meta_sb[:, B:2 * B]
            maskl_sb = meta_sb[:, 2 * B:3 * B]   # mask * (BS-1)

            negsum_all = fin_pool.tile([P, B], F32)
            t3m_all = fin_pool.tile([P, B], F32)
            terms_sb = fin_pool.tile([P, B], F32)
            ss_all = fin_pool.tile([P, B], F32)
            csr_all = fin_pool.tile([P, B], F32)

            for b in range(B):
                lhs = lhst_sb[:, b * KM:(b + 1) * KM, :]

                # full-row cosine chunks + exp row-sums
                rsp = sml_pool.tile([P, NG], F32, name=f"rsp{b}")
                for g in range(NG):
                    ps = psm_pool.tile([P, CW], F32, tag="psbig", bufs=2)
                    for n in range(CW // 512):
                        nc.tensor.matmul(
                            ps[:, n * 512:(n + 1) * 512],
                            lhs,
                            ent_sb[:, g * KM:(g + 1) * KM, n * 512:(n + 1) * 512],
                            start=True,
                            stop=True,
                            perf_mode=DR,
                        )
                    es = scr_pool.tile([P, CW], BF16, tag="escr", bufs=2)
                    nc.scalar.activation(
                        es[:], ps[:], AF.Exp,
                        accum_out=rsp[:, g:g + 1], scale=1.0 / CSC,
                    )

                # same-label range (zero-padded to W_s)
                ps_s = psm_pool.tile([P, CW], F32, tag="psbig", bufs=2)
                for h in range(WH):
                    nc.tensor.matmul(
                        ps_s[:, h * 512:(h + 1) * 512],
                        lhs,
                        rs_sb[:, b * KM:(b + 1) * KM, h * 512:(h + 1) * 512],
                        start=True,
                        stop=True,
                        perf_mode=DR,
                    )
                es_s = scr_pool.tile([P, CW], BF16, tag="escr", bufs=2)
                nc.scalar.activation(
                    es_s[:, :W_s], ps_s[:, :W_s], AF.Exp,
                    accum_out=ss_all[:, b:b + 1], scale=1.0 / CSC,
                )
                nc.vector.reduce_sum(csr_all[:, b:b + 1], ps_s[:, :W_s], axis=AX)

                # negsum = rs_all - ss + pad; everything except the batched
                # ln(negsum) is finished here on DVE, behind the Exp stream:
                # t3m = ((ss + C0)/negsum - csr/CSC + 1) * mask
                rs_a = sml_pool.tile([P, 1], F32, name=f"rsa{b}")
                nc.vector.reduce_sum(rs_a[:], rsp[:], axis=AX)
                nc.vector.tensor_sub(rs_a[:], rs_a[:], ss_all[:, b:b + 1])
                nc.vector.tensor_add(
                    negsum_all[:, b:b + 1], rs_a[:], pad_sb[:, b:b + 1])
                ub = sml_pool.tile([P, 1], F32, name=f"ub{b}")
                nc.vector.reciprocal(ub[:], negsum_all[:, b:b + 1])
                t1 = sml_pool.tile([P, 1], F32, name=f"t1_{b}")
                nc.vector.tensor_scalar(
                    t1[:], ss_all[:, b:b + 1], C0, ub[:], ADD, MUL)
                t2 = sml_pool.tile([P, 1], F32, name=f"t2_{b}")
                nc.vector.scalar_tensor_tensor(
                    t2[:], csr_all[:, b:b + 1], -1.0 / CSC, t1[:], MUL, ADD)
                nc.vector.tensor_scalar(
                    t3m_all[:, b:b + 1], t2[:], 1.0, mask_sb[:, b:b + 1],
                    ADD, MUL)

            # batched Ln (single Exp->Ln table switch), then one fused DVE op
            # per block: terms = ln(negsum) * (BS-1)*mask + t3m
            l_all = fin_pool.tile([P, B], F32)
            nc.scalar.activation(l_all[:], negsum_all[:], AF.Ln)
            for b in range(B):
                nc.vector.scalar_tensor_tensor(
                    terms_sb[:, b:b + 1], l_all[:, b:b + 1],
                    maskl_sb[:, b:b + 1], t3m_all[:, b:b + 1], MUL, ADD)

            nc.sync.dma_start(terms_d.ap(), terms_sb[:])

    nc.compile()
    return nc


# --------------------------------------------------------------------------
# Host orchestration
# --------------------------------------------------------------------------
def _plan_blocks(labels_s):
    counts = np.bincount(labels_s.astype(np.int64), minlength=L)
    starts = np.concatenate([[0], np.cumsum(counts)[:-1]])
    blocks = []
    for lab in range(L):
        s, c = int(starts[lab]), int(counts[lab])
        for off in range(0, c, P):
            blocks.append((s + off, min(P, c - off), lab))
    B = math.ceil(len(blocks) / NC)
    W_s = max(512, math.ceil((int(counts.max()) if len(blocks) else 1) / 512) * 512)
    return blocks, counts, starts, B, W_s


def _prep_launch_a_inputs(emb_s, W, b, label_emb):
    # embT k-tile-major: [P, KT, rows]
    embt_all = np.ascontiguousarray(
        emb_s.T.reshape(KT, P, BS).transpose(1, 0, 2))
    w8 = np.ascontiguousarray(
        (W.T * W_SCALE).reshape(KT, P, D_EMB).transpose(1, 0, 2)
    ).astype(F8_NP).reshape(P, KT * D_EMB)
    brow = (b * W_SCALE).reshape(1, KM * P).astype(F8_NP)
    ln = (label_emb / np.maximum(
        np.sqrt((label_emb.astype(np.float64) ** 2).sum(-1, keepdims=True)), 1e-8
    )).astype(np.float32)
    lnt8 = np.ascontiguousarray(
        (ln.T * EN_SCALE).reshape(KM, P, L).transpose(1, 0, 2)
    ).astype(F8_NP).reshape(P, KM * L)
    in_maps = []
    for c in range(NC):
        in_maps.append({
            "embt": np.ascontiguousarray(
                embt_all[:, :, c * RPC:(c + 1) * RPC]).reshape(P, KT * RPC),
            "wt": w8,
            "brow": brow,
            "lnt": lnt8,
        })
    return in_maps


def _prep_launch_b_inputs(entT_flat, blocks, counts, starts, B, W_s):
    """entT_flat: [P, KM, BS] f8 (= 16*en.T, partition-major)."""
    ent = np.ascontiguousarray(
        entT_flat.reshape(P, KM, NG, CW).transpose(0, 2, 1, 3)
    ).reshape(P, NG * KM * CW)
    in_maps = []
    for c in range(NC):
        blks = blocks[c * B:(c + 1) * B]
        lhst = np.zeros((P, B * KM, P), F8_NP)
        rsame = np.zeros((P, B * KM, W_s), F8_NP)
        meta = np.zeros((P, 3 * B), np.float32)
        for i, (rs, w, lab) in enumerate(blks):
            s, cnt = int(starts[lab]), int(counts[lab])
            for m in range(KM):
                lhst[:, i * KM + m, :w] = entT_flat[:, m, rs:rs + w]
                rsame[:, i * KM + m, :cnt] = entT_flat[:, m, s:s + cnt]
            meta[:w, i] = W_s - cnt            # pad
            meta[:w, B + i] = 1.0              # mask
            meta[:w, 2 * B + i] = float(BS - 1)  # mask * (BS-1)
        in_maps.append({
            "ent": ent,
            "lhst": lhst.reshape(P, B * KM * P),
            "rsame": rsame.reshape(P, B * KM * W_s),
            "meta": meta,
        })
    return in_maps


def _finalize_l1_l2(S_sorted, labels_s):
    S = S_sorted.astype(np.float64)
    idx = np.arange(BS)
    lab = labels_s.astype(np.int64)
    Pv = S[idx, lab]
    E2 = np.exp(S)
    eP = np.exp(Pv)
    neg1 = E2.sum(axis=1) - eP
    col_tot = E2.sum(axis=0)
    own_col = np.bincount(lab, weights=eP, minlength=L)
    neg2 = (col_tot - own_col)[lab]
    l1 = np.mean(-Pv + np.log(neg1 + eP))
    l2 = np.mean(-Pv + np.log(neg2 + eP))
    return l1, l2


def kernel(embedding, labels, W, b, label_emb):
    embedding = np.asarray(embedding, np.float32)
    labels_np = np.asarray(labels)
    W = np.asarray(W, np.float32)
    b = np.asarray(b, np.float32)
    label_emb = np.asarray(label_emb, np.float32)

    perm = np.argsort(labels_np, kind="stable")
    labels_s = labels_np[perm]
    emb_s = embedding[perm]
    blocks, counts, starts, B, W_s = _plan_blocks(labels_s)

    # ---- launch A ----
    nc_a = build_launch_a()
    in_maps_a = _prep_launch_a_inputs(emb_s, W, b, label_emb)
    res_a = run_bass_kernel_spmd(nc_a, in_maps_a, core_ids=list(range(NC)))
    LAST["a"] = res_a

    entT_flat = np.empty((P, KM, BS), F8_NP)
    S_sorted = np.empty((BS, L), np.float32)
    for c in range(NC):
        out = res_a.results[c]
        entT_flat[:, :, c * RPC:(c + 1) * RPC] = \
            np.asarray(out["ent_out"]).reshape(P, KM, RPC)
        s_c = np.asarray(out["s_out"]).reshape(P, RT, L)
        S_sorted[c * RPC:(c + 1) * RPC] = \
            s_c.transpose(1, 0, 2).reshape(RPC, L) / CSC

    # ---- launch B ----
    nc_b = build_launch_b(B, W_s)
    in_maps_b = _prep_launch_b_inputs(entT_flat, blocks, counts, starts, B, W_s)
    res_b = run_bass_kernel_spmd(nc_b, in_maps_b, core_ids=list(range(NC)))
    LAST["b"] = res_b

    total = 0.0
    for c in range(NC):
        total += np.asarray(res_b.results[c]["terms"], np.float64).sum()
    inter = total / (BS * BS)

    l1, l2 = _finalize_l1_l2(S_sorted, labels_s)
    return np.float32(0.5 * inter + 0.5 * (l1 + l2))


# revision 5
# speedup vs baseline: 1.0179x; 1.0179x over previous
"""Contrastive loss kernel for Trainium2 (8 NeuronCores, SPMD via bass).

Strategy (v2 — fp8 DoubleRow + first-order log expansion):
  * Host sorts the batch by label (loss is invariant under a joint row/col
    permutation); same-label columns become one contiguous range per label.
  * Launch A (data-parallel over rows): host supplies embT (k-tile-major
    f32, cast to fp8 in the DMA) and fp8 weights (x64). fp8 DoubleRow
    matmuls (K=256 per instruction at 2 cols/cycle) compute
    psE = 64*(emb @ W.T) + 64*b (bias via a k=1 fp8 matmul), norms via a
    ones-matmul over bf16 squares, and emit ent = 16*normalize(e).T as fp8
    plus S = en @ lnT (fp8 DoubleRow, x256 scale).
  * Launch B: all cores load the assembled full ent [256, 8192] (fp8,
    chunk-grouped [g][m][2048]). Each core owns up to B label-pure row
    blocks (<=128 rows). Per block: 16 fp8 DoubleRow matmuls produce the
    [128, 8192] cosine row in four 2048-col PSUM chunks; one ACT Exp per
    chunk (accum_out -> row sums). The same-label range is a zero-padded
    fp8 copy (width W_s): 2 DoubleRow matmuls + one ACT Exp accum give ss;
    a DVE reduce over its PSUM gives csr = sum(C).
  * exp(C)/negsum ~ 1e-4, so ln(negsum + y) = ln(negsum) + y/negsum to
    first order (error ~1e-9 on the final loss). The inter-sample term
    collapses to
      term = ((BS-1)*ln(negsum) + (ss + BS - W_s - e)/negsum + 1 - csr)*mask
    (zero-pad corrections fold into the global constant BS - W_s - e).
    Everything except the (BS-1)*ln(negsum) part is assembled per block on
    DVE behind the Exp stream; one batched ACT Ln on [128, B] per core plus
    one fused DVE op per block finishes the terms (a single Exp->Ln
    activation-table switch per core).
  * Host: inter = sum(terms)/bs^2; l1/l2 finalized from S in float64.
"""

import math
import os

import ml_dtypes
import numpy as np

os.environ.setdefault("NEURON_RT_VIRTUAL_CORE_SIZE", "1")

import concourse.bass as bass
import concourse.mybir as mybir
from concourse import bacc
import concourse.tile as tile
from concourse.bass_utils import run_bass_kernel_spmd

BS = 8192
D_IN = 1024
D_EMB = 256
L = 10
NC = 8
P = 128
RPC = BS // NC          # rows per core in launch A (1024)
RT = RPC // P           # 128-row tiles per core (8)
KT = D_IN // P          # k tiles (8)
KM = D_EMB // P         # emb-dim partition chunks (2)
CW = 2048               # psum chunk width (4 banks)
NG = BS // CW           # chunks per cosine row (4)

F32 = mybir.dt.float32
BF16 = mybir.dt.bfloat16
F8 = mybir.dt.float8e4
BF16_NP = ml_dtypes.bfloat16
F8_NP = ml_dtypes.float8_e4m3
W_SCALE = 64.0          # fp8 weight scale in launch A
EN_SCALE = 16.0         # ent = EN_SCALE * normalize(e).T
CSC = EN_SCALE * EN_SCALE   # cosine-psum scale (256)
AX = mybir.AxisListType.X
AF = mybir.ActivationFunctionType
DR = mybir.MatmulPerfMode.DoubleRow
MUL = mybir.AluOpType.mult
ADD = mybir.AluOpType.add

# Results of the last kernel() call (for test.py introspection/timing).
LAST = {}


# --------------------------------------------------------------------------
# Launch A: per-core transform  -> ent_out[128, KM*RPC] f8 (16*en.T), s_out
# --------------------------------------------------------------------------
def build_launch_a():
    nc = bacc.Bacc("TRN2", target_bir_lowering=False, debug=False, num_devices=NC)
    embt_d = nc.dram_tensor("embt", [P, KT * RPC], F32, kind="ExternalInput")
    wt_d = nc.dram_tensor("wt", [P, KT * D_EMB], F8, kind="ExternalInput")
    brow_d = nc.dram_tensor("brow", [1, KM * P], F8, kind="ExternalInput")
    lnt_d = nc.dram_tensor("lnt", [P, KM * L], F8, kind="ExternalInput")
    ent_d = nc.dram_tensor("ent_out", [P, KM * RPC], F8, kind="ExternalOutput")
    s_d = nc.dram_tensor("s_out", [P, RT * L], F32, kind="ExternalOutput")

    with tile.TileContext(nc) as tc:
        with (
            tc.tile_pool(name="const", bufs=1) as cpool,
            tc.tile_pool(name="big", bufs=1) as big_pool,
            tc.tile_pool(name="ps", bufs=1, space="PSUM") as ps_pool,
        ):
            embt_sb = big_pool.tile([P, KT, RPC], F8)
            # first embt chunk on the SWDGE queue before anything else; the
            # small fp8 constants ride the SP HWDGE queue in parallel
            nc.gpsimd.dma_start(
                embt_sb[:, 0:2, :], embt_d.ap()[:, 0:2 * RPC])
            wt_sb = cpool.tile([P, KT, D_EMB], F8)
            nc.sync.dma_start(wt_sb[:, :, :], wt_d.ap())
            brow_sb = cpool.tile([1, KM, P], F8)
            nc.sync.dma_start(brow_sb[:, :, :], brow_d.ap())
            lnt_sb = cpool.tile([P, KM, L], F8)
            nc.sync.dma_start(lnt_sb[:, :, :], lnt_d.ap())
            ones_row = cpool.tile([1, 512], F8)
            nc.vector.memset(ones_row[:], 1.0)
            ones_col = cpool.tile([P, 1], BF16)
            nc.vector.memset(ones_col[:], 1.0)
            # psb = (EN_SCALE / W_SCALE) / norm, constant folded into the
            # partition-broadcast matmul below
            onesq = cpool.tile([1, P], BF16)
            nc.vector.memset(onesq[:], EN_SCALE / W_SCALE)
            # dummy sqrt up front pins the sqrt_and_others act table (which
            # also serves Square and Copy) so no reload lands mid-chain
            dumm = cpool.tile([1, 1], F32)
            nc.vector.memset(dumm[:], 1.0)
            nc.scalar.sqrt(dumm[:], dumm[:])

            psE = [ps_pool.tile([P, RPC], F32, name=f"psE{m}") for m in range(KM)]

            # stream k-pairs: DMA chunk kk (f32->f8 cast in flight), then
            # accumulate fp8 DoubleRow matmuls (K=256 per instruction)
            for kk in range(KT // 2):
                if kk > 0:
                    nc.gpsimd.dma_start(
                        embt_sb[:, 2 * kk:2 * kk + 2, :],
                        embt_d.ap()[:, 2 * kk * RPC:(2 * kk + 2) * RPC],
                    )
                for m in range(KM):
                    for n in range(RPC // 512):
                        nc.tensor.matmul(
                            psE[m][:, n * 512:(n + 1) * 512],
                            wt_sb[:, 2 * kk:2 * kk + 2, m * P:(m + 1) * P],
                            embt_sb[:, 2 * kk:2 * kk + 2, n * 512:(n + 1) * 512],
                            start=(kk == 0),
                            stop=False,
                            perf_mode=DR,
                        )
            # bias rows (k=1 fp8): psE = W_SCALE * (emb @ W.T + b)
            for m in range(KM):
                for n in range(RPC // 512):
                    nc.tensor.matmul(
                        psE[m][:, n * 512:(n + 1) * 512],
                        brow_sb[:, m, :],
                        ones_row[:],
                        start=False,
                        stop=True,
                    )

            # norm chain, split into 512-col halves so ACT/PE/DVE pipeline:
            # esq = (psE/W_SCALE)^2 (bf16) -> psN = colsum -> sqrt -> 1/x ->
            # psB = (EN_SCALE/W_SCALE)/norm broadcast -> ent = psE * psB (f8)
            NH = RPC // 512
            esq = big_pool.tile([P, KM * RPC], BF16)
            rn = big_pool.tile([1, RPC], F32)
            rni = big_pool.tile([1, RPC], BF16)
            ent_sb = big_pool.tile([P, KM, RPC], F8)
            psB = [ps_pool.tile([P, 512], F32, name=f"psB{n}") for n in range(NH)]
            psN = [ps_pool.tile([1, 512], F32, tag="aux", bufs=2, name=f"psN{n}")
                   for n in range(NH)]
            sbB = [big_pool.tile([P, 512], BF16, name=f"sbB{n}") for n in range(NH)]
            # phase-ordered so no engine stalls behind a later dependency:
            # ACT: squares -> sqrts -> psB copies; PE: psN -> psB; DVE:
            # recips -> ent muls
            for n in range(NH):
                for m in range(KM):
                    nc.scalar.activation(
                        esq[:, m * RPC + n * 512: m * RPC + (n + 1) * 512],
                        psE[m][:, n * 512:(n + 1) * 512],
                        AF.Square, scale=1.0 / W_SCALE,
                    )
            for n in range(NH):
                for m in range(KM):
                    nc.tensor.matmul(
                        psN[n][:],
                        ones_col[:],
                        esq[:, m * RPC + n * 512: m * RPC + (n + 1) * 512],
                        start=(m == 0),
                        stop=(m == KM - 1),
                    )
            for n in range(NH):
                nc.scalar.sqrt(rn[:, n * 512:(n + 1) * 512], psN[n][:])
            with nc.allow_low_precision(reason="1/norm feeds fp8 output"):
                for n in range(NH):
                    nc.vector.reciprocal(
                        rni[:, n * 512:(n + 1) * 512],
                        rn[:, n * 512:(n + 1) * 512])
            for n in range(NH):
                nc.tensor.matmul(
                    psB[n][:], onesq[:], rni[:, n * 512:(n + 1) * 512],
                    start=True, stop=True)
            for n in range(NH):
                # DVE can read only one PSUM operand: stage psB in SBUF
                nc.scalar.activation(sbB[n][:], psB[n][:], AF.Copy)
            for n in range(NH):
                for m in range(KM):
                    nc.vector.tensor_mul(
                        ent_sb[:, m, n * 512:(n + 1) * 512],
                        psE[m][:, n * 512:(n + 1) * 512], sbB[n][:])
            nc.sync.dma_start(ent_d.ap(), ent_sb[:, :, :])

            # S = en @ lnT (fp8 DoubleRow; psS = CSC * S)
            psS = ps_pool.tile([P, RT * L], F32, tag="aux", bufs=2)
            for r in range(RT):
                nc.tensor.matmul(
                    psS[:, r * L:(r + 1) * L],
                    ent_sb[:, 0:KM, r * P:(r + 1) * P],
                    lnt_sb[:, 0:KM, :],
                    start=True,
                    stop=True,
                    perf_mode=DR,
                )
            s_sb = big_pool.tile([P, RT * L], F32)
            nc.vector.tensor_copy(s_sb[:], psS[:])
            nc.gpsimd.dma_start(s_d.ap(), s_sb[:])

    nc.compile()
    return nc


# --------------------------------------------------------------------------
# Launch B: B label-pure block slots of the inter-sample loss per core
# --------------------------------------------------------------------------
def build_launch_b(B, W_s):
    WH = W_s // 512
    C0 = float(BS - W_s - math.e)
    nc = bacc.Bacc("TRN2", target_bir_lowering=False, debug=False, num_devices=NC)
    ent_d = nc.dram_tensor("ent", [P, NG * KM * CW], F8, kind="ExternalInput")
    lhst_d = nc.dram_tensor("lhst", [P, B * KM * P], F8, kind="ExternalInput")
    rs_d = nc.dram_tensor("rsame", [P, B * KM * W_s], F8, kind="ExternalInput")
    meta_d = nc.dram_tensor("meta", [P, 3 * B], F32, kind="ExternalInput")
    terms_d = nc.dram_tensor("terms", [P, B], F32, kind="ExternalOutput")

    with tile.TileContext(nc) as tc:
        with (
            tc.tile_pool(name="inp", bufs=1) as inp_pool,
            tc.tile_pool(name="scr", bufs=2) as scr_pool,
            tc.tile_pool(name="sml", bufs=2) as sml_pool,
            tc.tile_pool(name="fin", bufs=1) as fin_pool,
            tc.tile_pool(name="psm", bufs=2, space="PSUM") as psm_pool,
        ):
            ent_sb = inp_pool.tile([P, NG * KM, CW], F8)
            lhst_sb = inp_pool.tile([P, B * KM, P], F8)
            rs_sb = inp_pool.tile([P, B * KM, W_s], F8)
            meta_sb = inp_pool.tile([P, 3 * B], F32)
            # SP HWDGE queue in dependency order: block-0 lhs and the ent
            # chunks first; rsame streams behind in per-2-block pieces on
            # the SWDGE queue so it never overtakes the ent chunks; meta is
            # only needed at block-0 wrap-up
            nc.gpsimd.dma_start(lhst_sb[:, 0:KM, :], lhst_d.ap()[:, 0:KM * P])
            for g in range(NG):
                nc.sync.dma_start(
                    ent_sb[:, g * KM:(g + 1) * KM, :],
                    ent_d.ap()[:, g * KM * CW:(g + 1) * KM * CW],
                )
            if B > 1:
                nc.sync.dma_start(
                    lhst_sb[:, KM:B * KM, :], lhst_d.ap()[:, KM * P:])
            nc.sync.dma_start(meta_sb[:], meta_d.ap())
            for b0 in range(0, B, 2):
                b1 = min(b0 + 2, B)
                nc.gpsimd.dma_start(
                    rs_sb[:, b0 * KM:b1 * KM, :],
                    rs_d.ap()[:, b0 * KM * W_s:b1 * KM * W_s])
            pad_sb = meta_sb[:, 0:B]
            mask_sb = meta_sb[:, B:2 * B]
            maskl_sb = meta_sb[:, 2 * B:3 * B]   # mask * (BS-1)

            negsum_all = fin_pool.tile([P, B], F32)
            t3m_all = fin_pool.tile([P, B], F32)
            terms_sb = fin_pool.tile([P, B], F32)
            ss_all = fin_pool.tile([P, B], F32)
            csr_all = fin_pool.tile([P, B], F32)

            for b in range(B):
                lhs = lhst_sb[:, b * KM:(b + 1) * KM, :]

                # full-row cosine chunks + exp row-sums
                rsp = sml_pool.tile([P, NG], F32, name=f"rsp{b}")
                for g in range(NG):
                    ps = psm_pool.tile([P, CW], F32, tag="psbig", bufs=2)
                    for n in range(CW // 512):
                        nc.tensor.matmul(
                            ps[:, n * 512:(n + 1) * 512],
                            lhs,
                            ent_sb[:, g * KM:(g + 1) * KM, n * 512:(n + 1) * 512],
                            start=True,
                            stop=True,
                            perf_mode=DR,
                        )
                    es = scr_pool.tile([P, CW], BF16, tag="escr", bufs=2)
                    nc.scalar.activation(
                        es[:], ps[:], AF.Exp,
                        accum_out=rsp[:, g:g + 1], scale=1.0 / CSC,
                    )

                # same-label range (zero-padded to W_s)
                ps_s = psm_pool.tile([P, CW], F32, tag="psbig", bufs=2)
                for h in range(WH):
                    nc.tensor.matmul(
                        ps_s[:, h * 512:(h + 1) * 512],
                        lhs,
                        rs_sb[:, b * KM:(b + 1) * KM, h * 512:(h + 1) * 512],
                        start=True,
                        stop=True,
                        perf_mode=DR,
                    )
                es_s = scr_pool.tile([P, CW], BF16, tag="escr", bufs=2)
                nc.scalar.activation(
                    es_s[:, :W_s], ps_s[:, :W_s], AF.Exp,
                    accum_out=ss_all[:, b:b + 1], scale=1.0 / CSC,
                )
                nc.vector.reduce_sum(csr_all[:, b:b + 1], ps_s[:, :W_s], axis=AX)

                # negsum = rs_all - ss + pad; everything except the batched
                # ln(negsum) is finished here on DVE, behind the Exp stream:
                # t3m = ((ss + C0)/negsum - csr/CSC + 1) * mask
                rs_a = sml_pool.tile([P, 1], F32, name=f"rsa{b}")
                nc.vector.reduce_sum(rs_a[:], rsp[:], axis=AX)
                nc.vector.tensor_sub(rs_a[:], rs_a[:], ss_all[:, b:b + 1])
                nc.vector.tensor_add(
                    negsum_all[:, b:b + 1], rs_a[:], pad_sb[:, b:b + 1])
                ub = sml_pool.tile([P, 1], F32, name=f"ub{b}")
                nc.vector.reciprocal(ub[:], negsum_all[:, b:b + 1])
                t1 = sml_pool.tile([P, 1], F32, name=f"t1_{b}")
                nc.vector.tensor_scalar(
                    t1[:], ss_all[:, b:b + 1], C0, ub[:], ADD, MUL)
                t2 = sml_pool.tile([P, 1], F32, name=f"t2_{b}")
                nc.vector.scalar_tensor_tensor(
                    t2[:], csr_all[:, b:b + 1], -1.0 / CSC, t1[:], MUL, ADD)
                nc.vector.tensor_scalar(
                    t3m_all[:, b:b + 1], t2[:], 1.0, mask_sb[:, b:b + 1],
                    ADD, MUL)

            # batched Ln (single Exp->Ln table switch), then one fused DVE op
            # per block: terms = ln(negsum) * (BS-1)*mask + t3m
            l_all = fin_pool.tile([P, B], F32)
            nc.scalar.activation(l_all[:], negsum_all[:], AF.Ln)
            for b in range(B):
                nc.vector.scalar_tensor_tensor(
                    terms_sb[:, b:b + 1], l_all[:, b:b + 1],
                    maskl_sb[:, b:b + 1], t3m_all[:, b:b + 1], MUL, ADD)

            nc.sync.dma_start(terms_d.ap(), terms_sb[:])

    nc.compile()
    return nc


# --------------------------------------------------------------------------
# Host orchestration
# --------------------------------------------------------------------------
def _plan_blocks(labels_s):
    counts = np.bincount(labels_s.astype(np.int64), minlength=L)
    starts = np.concatenate([[0], np.cumsum(counts)[:-1]])
    blocks = []
    for lab in range(L):
        s, c = int(starts[lab]), int(counts[lab])
        for off in range(0, c, P):
            blocks.append((s + off, min(P, c - off), lab))
    B = math.ceil(len(blocks) / NC)
    W_s = max(512, math.ceil((int(counts.max()) if len(blocks) else 1) / 512) * 512)
    return blocks, counts, starts, B, W_s


def _prep_launch_a_inputs(emb_s, W, b, label_emb):
    # embT k-tile-major: [P, KT, rows]
    embt_all = np.ascontiguousarray(
        emb_s.T.reshape(KT, P, BS).transpose(1, 0, 2))
    w8 = np.ascontiguousarray(
        (W.T * W_SCALE).reshape(KT, P, D_EMB).transpose(1, 0, 2)
    ).astype(F8_NP).reshape(P, KT * D_EMB)
    brow = (b * W_SCALE).reshape(1, KM * P).astype(F8_NP)
    ln = (label_emb / np.maximum(
        np.sqrt((label_emb.astype(np.float64) ** 2).sum(-1, keepdims=True)), 1e-8
    )).astype(np.float32)
    lnt8 = np.ascontiguousarray(
        (ln.T * EN_SCALE).reshape(KM, P, L).transpose(1, 0, 2)
    ).astype(F8_NP).reshape(P, KM * L)
    in_maps = []
    for c in range(NC):
        in_maps.append({
            "embt": np.ascontiguousarray(
                embt_all[:, :, c * RPC:(c + 1) * RPC]).reshape(P, KT * RPC),
            "wt": w8,
            "brow": brow,
            "lnt": lnt8,
        })
    return in_maps


def _prep_launch_b_inputs(entT_flat, blocks, counts, starts, B, W_s):
    """entT_flat: [P, KM, BS] f8 (= 16*en.T, partition-major)."""
    ent = np.ascontiguousarray(
        entT_flat.reshape(P, KM, NG, CW).transpose(0, 2, 1, 3)
    ).reshape(P, NG * KM * CW)
    in_maps = []
    for c in range(NC):
        blks = blocks[c * B:(c + 1) * B]
        lhst = np.zeros((P, B * KM, P), F8_NP)
        rsame = np.zeros((P, B * KM, W_s), F8_NP)
        meta = np.zeros((P, 3 * B), np.float32)
        for i, (rs, w, lab) in enumerate(blks):
            s, cnt = int(starts[lab]), int(counts[lab])
            for m in range(KM):
                lhst[:, i * KM + m, :w] = entT_flat[:, m, rs:rs + w]
                rsame[:, i * KM + m, :cnt] = entT_flat[:, m, s:s + cnt]
            meta[:w, i] = W_s - cnt            # pad
            meta[:w, B + i] = 1.0              # mask
            meta[:w, 2 * B + i] = float(BS - 1)  # mask * (BS-1)
        in_maps.append({
            "ent": ent,
            "lhst": lhst.reshape(P, B * KM * P),
            "rsame": rsame.reshape(P, B * KM * W_s),
            "meta": meta,
        })
    return in_maps


def _finalize_l1_l2(S_sorted, labels_s):
    S = S_sorted.astype(np.float64)
    idx = np.arange(BS)
    lab = labels_s.astype(np.int64)
    Pv = S[idx, lab]
    E2 = np.exp(S)
    eP = np.exp(Pv)
    neg1 = E2.sum(axis=1) - eP
    col_tot = E2.sum(axis=0)
    own_col = np.bincount(lab, weights=eP, minlength=L)
    neg2 = (col_tot - own_col)[lab]
    l1 = np.mean(-Pv + np.log(neg1 + eP))
    l2 = np.mean(-Pv + np.log(neg2 + eP))
    return l1, l2


def kernel(embedding, labels, W, b, label_emb):
    embedding = np.asarray(embedding, np.float32)
    labels_np = np.asarray(labels)
    W = np.asarray(W, np.float32)
    b = np.asarray(b, np.float32)
    label_emb = np.asarray(label_emb, np.float32)

    perm = np.argsort(labels_np, kind="stable")
    labels_s = labels_np[perm]
    emb_s = embedding[perm]
    blocks, counts, starts, B, W_s = _plan_blocks(labels_s)

    # ---- launch A ----
    nc_a = build_launch_a()
    in_maps_a = _prep_launch_a_inputs(emb_s, W, b, label_emb)
    res_a = run_bass_kernel_spmd(nc_a, in_maps_a, core_ids=list(range(NC)))
    LAST["a"] = res_a

    entT_flat = np.empty((P, KM, BS), F8_NP)
    S_sorted = np.empty((BS, L), np.float32)
    for c in range(NC):
        out = res_a.results[c]
        entT_flat[:, :, c * RPC:(c + 1) * RPC] = \
            np.asarray(out["ent_out"]).reshape(P, KM, RPC)
        s_c = np.asarray(out["s_out"]).reshape(P, RT, L)
        S_sorted[c * RPC:(c + 1) * RPC] = \
            s_c.transpose(1, 0, 2).reshape(RPC, L) / CSC

    # ---- launch B ----
    nc_b = build_launch_b(B, W_s)
    in_maps_b = _prep_launch_b_inputs(entT_flat, blocks, counts, starts, B, W_s)
    res_b = run_bass_kernel_spmd(nc_b, in_maps_b, core_ids=list(range(NC)))
    LAST["b"] = res_b

    total = 0.0
    for c in range(NC):
        total += np.asarray(res_b.results[c]["terms"], np.float64).sum()
    inter = total / (BS * BS)

    l1, l2 = _finalize_l1_l2(S_sorted, labels_s)
    return np.float32(0.5 * inter + 0.5 * (l1 + l2))
